# revision 30
# baseline (speedup 1.0000x reference)
"""Expert-parallel MoE feed-forward for Trainium2 (8 NeuronCores).

Strategy:
  - Host: gate + top-2 routing (0.02% of FLOPs), per-expert token lists,
    gather + transpose + bf16-cast of each expert's tokens -> xT [D, C].
    Expert e is owned by core e.  C = max expert load, rounded up to 8
    (1080 for the reference input); the program is compiled per C and
    cached.  All device inputs are pre-swizzled partition-major on the
    host so every DMA is one large 2D transfer with multi-KB lines.
  - Device (same SPMD program on all 8 cores), all matmuls bf16 (fp32
    PSUM accumulate; rel err ~3e-3 << 2e-2 tolerance):
      mm1: h[f, t]  = relu(W1[k,f-slice].T @ xT[k, t] + b1)   (acc over k)
      mm2: yT[d, t] = W2[j,d-slice].T @ h[j, t] + b2          (acc over j)
      yT *= wc[t];  store yT [D, C] fp32.
    No device gather / transposes: PE runs a pure matmul stream
    (~2*256*C rows; bf16 moving streams ~2 elem/cycle).
  - Weight-load hiding: matmuls sharing a stationary tile are emitted
    consecutively (walrus dedupes consecutive identical weight loads),
    and the next tile is pre-issued via nc.tensor.ldweights between
    accumulation steps (ldw="inner") so the PE reorder window loads it
    during the current matmuls: measured 267us -> 194us (uncapped C).
    Early wedge suspicion about this mode was retired by a dedicated
    stress test: 40 single execs + 676 loop iterations, 0 hangs,
    correct output throughout (stress_ldw.py).
  - DMA triggers split across the two HWDGE issuing engines: SP carries
    the mm1-critical xt + W1 stream in consumption order, Activation
    carries W2 + y-out.  17 input triggers per iteration total; w1 ring
    3 deep, xt double-buffered for cross-iteration prefetch.
  - mm1 PSUM drain on DVE (tensor_scalar add-bias + max-0), mm2 drain on
    ScalarE (Identity + bias) -> drains run on different engines.
  - Host: transpose yT back, scatter-add into the [B,S,D] output.
"""

import numpy as np

B, S, D, F, E = 2, 2048, 1024, 4096, 8
T = B * S                      # 4096 tokens
K_TOP = 2
P = 128
NJ = F // P                    # 32 F-tiles
KD = D // P                    # 8  D-tiles
NJ4 = NJ // 4                  # 8  j4 groups (W1 stream granule)
W1COLS = NJ4 * KD * 512        # 16384  (pre-swizzled W1 free size)
W2COLS = NJ * D                # 32768  (pre-swizzled W2 free size)

_CACHE = {}


def _chunks(C, step=512):
    out, c0 = [], 0
    while c0 < C:
        cw = min(step, C - c0)
        out.append((c0, cw))
        c0 += cw
    return out


def _build_program(C, reps=1, loop_n=1, parts="all", ldw="inner"):
    # parts: "all" | "dma" (skip matmuls/acts) | "pe" (hoist input DMAs
    # out of the timing loop)  — probe modes for bottleneck isolation
    import concourse.bass as bass
    import concourse.mybir as mybir
    import concourse.tile as tile
    from concourse import bacc
    from contextlib import ExitStack

    f32 = mybir.dt.float32
    bf16 = mybir.dt.bfloat16

    nc = bacc.Bacc("TRN2", target_bir_lowering=False, debug=False)

    # host-swizzled layouts (see _mk_inmaps):
    #   xt_d[p, k*C + t]          = x[tok_t, k*128 + p]
    #   w1_d[p, (j4*KD + k)*512 + c] = W1[k*128 + p, j4*512 + c]
    #   w2_d[p, k*D + d]          = W2[k*128 + p, d]
    xt_d = nc.dram_tensor("xt", [P, KD * C], bf16, kind="ExternalInput").ap()
    w1_d = nc.dram_tensor("W1r", [P, W1COLS], bf16, kind="ExternalInput").ap()
    w2_d = nc.dram_tensor("W2r", [P, W2COLS], bf16, kind="ExternalInput").ap()
    wcr_d = nc.dram_tensor("wcr", [P, C], f32, kind="ExternalInput").ap()
    b1_d = nc.dram_tensor("b1t", [P, NJ], f32, kind="ExternalInput").ap()
    b2_d = nc.dram_tensor("b2t", [P, KD], f32, kind="ExternalInput").ap()
    y_d = nc.dram_tensor("yout", [D, C], f32, kind="ExternalOutput").ap()

    CH = _chunks(C)
    NCH = len(CH)
    GK = KD * 512              # 4096 w1 cols per j4 group

    hoist_dma = (parts == "pe")
    skip_mm = (parts == "dma")

    with tile.TileContext(nc) as tc, ExitStack() as ctx:
        sb = ctx.enter_context(tc.tile_pool(name="sb", bufs=1))
        ps = ctx.enter_context(tc.tile_pool(name="ps", bufs=1, space="PSUM"))

        wcr_t = sb.tile([P, C], f32, tag="wcr")
        b1_t = sb.tile([P, NJ], f32, tag="b1")
        b2_t = sb.tile([P, KD], f32, tag="b2")
        nc.scalar.dma_start(wcr_t[:], wcr_d[:])
        nc.scalar.dma_start(b1_t[:], b1_d[:])
        nc.scalar.dma_start(b2_t[:], b2_d[:])

        def xt_tile(rep):
            xt = sb.tile([P, KD * C], bf16, tag="xt", bufs=2,
                         name=f"xt_{rep}")
            nc.sync.dma_start(xt[:], xt_d[:])
            return xt

        def w1_tile(rep, j4):
            w1t = sb.tile([P, GK], bf16, tag="w1", bufs=4,
                          name=f"w1_{rep}_{j4}")
            nc.sync.dma_start(w1t[:], w1_d[:, j4 * GK:(j4 + 1) * GK])
            return w1t

        def w2_tile(rep, kg):
            # 4 k-slices per trigger, on the Activation HWDGE queue
            w2t = sb.tile([P, 4 * D], bf16, tag="w2f", bufs=KD,
                          name=f"w2_{rep}_{kg}")
            nc.scalar.dma_start(w2t[:], w2_d[:, kg * 4 * D:(kg + 1) * 4 * D])
            return w2t

        if hoist_dma:
            xt0 = xt_tile(0)
            w2ts0 = [w2_tile(0, kg) for kg in range(KD)]
            w1ts0 = [w1_tile(0, j4) for j4 in range(3)]

        loop_cm = tc.For_i(0, loop_n, 1) if loop_n > 1 else None
        if loop_cm is not None:
            loop_cm.__enter__()

        for rep in range(reps):
            if hoist_dma:
                xt, w2ts = xt0, w2ts0
                w1tiles = [w1ts0[j4 % 3] for j4 in range(NJ4)]
            else:
                xt = xt_tile(rep)
                w2ts = [w2_tile(rep, kg) for kg in range(KD)]
                w1tiles = [w1_tile(rep, j4) for j4 in range(NJ4)]
            h = sb.tile([P, NJ * C], bf16, tag="h", name=f"h_{rep}")

            if skip_mm:
                dummy = sb.tile([P, C], f32, tag="ot", bufs=2,
                                name=f"dummy_{rep}")
                nc.vector.memset(dummy[:], 0.0)
                for dch in range(KD):
                    nc.scalar.dma_start(y_d[dch * P:(dch + 1) * P, :],
                                        dummy[:])
                continue

            # weight-tile APs in PE execution order; each group's stationary
            # tile is pre-issued (ldweights) during the previous group's
            # matmuls so the PE reorder window hides the load
            def w1ap(j4, jj, k):
                return w1tiles[j4][:, k * 512 + jj * P: k * 512 + (jj + 1) * P]

            def w2ap(dch, k):
                kg, k4 = divmod(k, 4)
                return w2ts[kg][:, k4 * D + dch * P: k4 * D + (dch + 1) * P]

            groups1 = [(j4, jj, k)
                       for j4 in range(NJ4) for jj in range(4)
                       for k in range(KD)]
            groups2 = [(dch, k) for dch in range(KD) for k in range(NJ)]
            nexts = ([w1ap(*g) for g in groups1[1:]] + [w2ap(*groups2[0])]
                     + [w2ap(*g) for g in groups2[1:]] + [None])
            gi = 0

            # --- mm1 + relu:  h[j*C + t] = relu(x @ W1 + b1) ---
            for j4 in range(NJ4):
                for jj in range(4):
                    j = j4 * 4 + jj
                    pcs = [ps.tile([P, 512], f32, tag="mm", bufs=8,
                                   name=f"p1_{rep}_{j}_{ci}")
                           for ci in range(NCH)]
                    for k in range(KD):
                        for ci, (c0, cw) in enumerate(CH):
                            nc.tensor.matmul(
                                pcs[ci][:, :cw],
                                lhsT=w1ap(j4, jj, k),
                                rhs=xt[:, k * C + c0: k * C + c0 + cw],
                                start=(k == 0), stop=(k == KD - 1))
                        if ldw and nexts[gi] is not None and (
                                ldw is True or k < KD - 1):
                            nc.tensor.ldweights(nexts[gi])
                        gi += 1
                    for ci, (c0, cw) in enumerate(CH):
                        # relu(psum + b1) on DVE: add bias then max(,0)
                        nc.vector.tensor_scalar(
                            out=h[:, j * C + c0: j * C + c0 + cw],
                            in0=pcs[ci][:, :cw],
                            scalar1=b1_t[:, j:j + 1], scalar2=0.0,
                            op0=mybir.AluOpType.add,
                            op1=mybir.AluOpType.max)

            # --- mm2:  yT[dch*128+p, t] = sum_j W2[j,d].T @ h[j, t] ---
            # dch outer, chunks innermost: consecutive matmuls share lhsT
            # (walrus dedupes consecutive identical weight loads)
            for dch in range(KD):
                pds = [ps.tile([P, 512], f32, tag="mm", bufs=8,
                               name=f"p2_{rep}_{dch}_{ci}")
                       for ci in range(NCH)]
                for k in range(NJ):
                    for ci, (c0, cw) in enumerate(CH):
                        nc.tensor.matmul(
                            pds[ci][:, :cw],
                            lhsT=w2ap(dch, k),
                            rhs=h[:, k * C + c0: k * C + c0 + cw],
                            start=(k == 0), stop=(k == NJ - 1))
                    if ldw and nexts[gi] is not None and (
                            ldw is True or k < NJ - 1):
                        nc.tensor.ldweights(nexts[gi])
                    gi += 1
                ot = sb.tile([P, C], f32, tag="ot", bufs=2,
                             name=f"ot_{rep}_{dch}")
                for ci, (c0, cw) in enumerate(CH):
                    nc.scalar.activation(
                        ot[:, c0:c0 + cw], pds[ci][:, :cw],
                        mybir.ActivationFunctionType.Identity,
                        bias=b2_t[:, dch:dch + 1])
                nc.vector.tensor_tensor(
                    out=ot[:], in0=ot[:], in1=wcr_t[:],
                    op=mybir.AluOpType.mult)
                nc.scalar.dma_start(y_d[dch * P:(dch + 1) * P, :], ot[:])

        if loop_cm is not None:
            loop_cm.__exit__(None, None, None)

    nc.compile()
    return nc


def _route(x2, Wg, bg):
    """Host-side top-2 routing in float64 (stable ordering)."""
    gate = x2.astype(np.float64) @ np.asarray(Wg, np.float64) + np.asarray(bg, np.float64)
    part = np.argpartition(-gate, K_TOP - 1, axis=1)[:, :K_TOP]      # [T, 2]
    rows = np.arange(T)[:, None]
    sc = gate[rows, part]                                            # [T, 2]
    sc = sc - sc.max(axis=1, keepdims=True)
    e_sc = np.exp(sc)
    probs = e_sc / e_sc.sum(axis=1, keepdims=True)                   # [T, 2]
    idx_e, w_e, n_e = [], [], []
    for e in range(E):
        mask = part == e                                             # [T, 2]
        tok = np.nonzero(mask.any(axis=1))[0]
        pr = probs[mask]                                             # aligned with tok
        idx_e.append(tok.astype(np.int32))
        w_e.append(pr.astype(np.float32))
        n_e.append(len(tok))
    return idx_e, w_e, n_e


CAP = 1024                     # device per-expert capacity (= T*K_TOP/E);
                               # overflow pairs (~1.5% of FLOPs worst case)
                               # are computed on the host in f32


def _cap_routing(idx_e, w_e, n_e):
    """Split routing into device part (first CAP tokens per expert) and
    host overflow list [(e, toks, wcs)]."""
    idx_c, w_c, n_c, over = [], [], [], []
    for e in range(E):
        n = n_e[e]
        if n > CAP:
            over.append((e, idx_e[e][CAP:], w_e[e][CAP:]))
        idx_c.append(idx_e[e][:CAP])
        w_c.append(w_e[e][:CAP])
        n_c.append(min(n, CAP))
    return idx_c, w_c, n_c, over


def _mk_inmaps(x2, W1, b1, W2, b2, idx_e, w_e, C):
    import ml_dtypes
    bf16 = ml_dtypes.bfloat16
    x2T = np.ascontiguousarray(x2.T)                     # [D, T]
    in_maps = []
    for e in range(E):
        idx = idx_e[e]
        n = len(idx)
        xT = np.zeros((KD, P, C), bf16)                  # [k, p, t]
        xT.reshape(D, C)[:, :n] = x2T[:, idx].astype(bf16)
        wc = np.zeros((C,), np.float32)
        wc[:n] = w_e[e]
        # swizzles: see _build_program layout comments
        w1r = np.ascontiguousarray(
            W1[e].astype(bf16).reshape(KD, P, NJ4, 512)
            .transpose(1, 2, 0, 3)).reshape(P, W1COLS)
        w2r = np.ascontiguousarray(
            W2[e].astype(bf16).reshape(NJ, P, D)
            .transpose(1, 0, 2)).reshape(P, W2COLS)
        in_maps.append({
            "xt": np.ascontiguousarray(xT.transpose(1, 0, 2)).reshape(P, KD * C),
            "W1r": w1r,
            "W2r": w2r,
            "wcr": np.ascontiguousarray(np.broadcast_to(wc, (P, C))),
            "b1t": np.ascontiguousarray(b1[e].reshape(NJ, P).T),
            "b2t": np.ascontiguousarray(b2[e].reshape(KD, P).T),
        })
    return in_maps


def kernel(x, W1, b1, W2, b2, Wg, bg, num_experts_per_token):
    from concourse.bass_utils import run_bass_kernel_spmd

    x2 = np.ascontiguousarray(np.asarray(x, np.float32).reshape(T, D))
    W1 = np.asarray(W1, np.float32)
    b1 = np.asarray(b1, np.float32)
    W2 = np.asarray(W2, np.float32)
    b2 = np.asarray(b2, np.float32)

    if int(num_experts_per_token) != K_TOP:
        # unexpected top-k: correct numpy slow path
        gate = x2.astype(np.float64) @ np.asarray(Wg, np.float64) + np.asarray(bg, np.float64)
        k = int(num_experts_per_token)
        part = np.argsort(-gate, axis=1)[:, :k]
        sc = gate[np.arange(T)[:, None], part]
        sc = sc - sc.max(axis=1, keepdims=True)
        pr = np.exp(sc); pr /= pr.sum(axis=1, keepdims=True)
        out = np.zeros((T, D), np.float32)
        for e in range(E):
            mask = part == e
            tok = np.nonzero(mask.any(axis=1))[0]
            w = pr[mask].astype(np.float32)
            hcur = np.maximum(x2[tok] @ W1[e] + b1[e], 0.0)
            out[tok] += w[:, None] * (hcur @ W2[e] + b2[e])
        return out.reshape(B, S, D)

    idx_e, w_e, n_e, over = _cap_routing(*_route(x2, Wg, bg))
    C = max((max(n_e) + 7) // 8 * 8, 64)

    if C not in _CACHE:
        _CACHE[C] = _build_program(C)
    nc = _CACHE[C]

    in_maps = _mk_inmaps(x2, W1, b1, W2, b2, idx_e, w_e, C)
    res = run_bass_kernel_spmd(nc, in_maps, list(range(E)))

    out = np.zeros((T, D), np.float32)
    for e in range(E):
        n = n_e[e]
        yT = res.results[e]["yout"]                      # [D, C] f32
        out[idx_e[e]] += yT[:, :n].T
    # host-side overflow pass (tokens beyond per-expert capacity)
    for e, toks, wcs in over:
        ho = np.maximum(x2[toks] @ W1[e] + b1[e], 0.0)
        out[toks] += wcs[:, None] * (ho @ W2[e] + b2[e])
    return out.reshape(B, S, D)


# revision 31
# speedup vs baseline: 1.1934x; 1.1934x over previous
"""Expert-parallel MoE feed-forward for Trainium2 (8 NeuronCores).

Strategy:
  - Host: gate + top-2 routing (0.02% of FLOPs), per-expert token lists,
    gather + transpose + bf16-cast of each expert's tokens -> xT [D, C].
    Expert e is owned by core e.  C = max expert load, rounded up to 8
    (1080 for the reference input); the program is compiled per C and
    cached.  All device inputs are pre-swizzled partition-major on the
    host so every DMA is one large 2D transfer with multi-KB lines.
  - Device (same SPMD program on all 8 cores), all matmuls bf16 (fp32
    PSUM accumulate; rel err ~3e-3 << 2e-2 tolerance):
      mm1: h[f, t]  = relu(W1[k,f-slice].T @ xT[k, t] + b1)   (acc over k)
      mm2: yT[d, t] = W2[j,d-slice].T @ h[j, t] + b2          (acc over j)
      yT *= wc[t];  store yT [D, C] fp32.
    No device gather / transposes: PE runs a pure matmul stream
    (~2*256*C rows; bf16 moving streams ~2 elem/cycle).
  - Weight-load hiding: matmuls sharing a stationary tile are emitted
    consecutively (walrus dedupes consecutive identical weight loads),
    and the next tile is pre-issued via nc.tensor.ldweights between
    accumulation steps (ldw="inner") so the PE reorder window loads it
    during the current matmuls: measured 267us -> 194us (uncapped C).
    Early wedge suspicion about this mode was retired by a dedicated
    stress test: 40 single execs + 676 loop iterations, 0 hangs,
    correct output throughout (stress_ldw.py).
  - DMA triggers split across the two HWDGE issuing engines: SP carries
    the mm1-critical xt + W1 stream in consumption order, Activation
    carries W2 + y-out.  17 input triggers per iteration total; w1 ring
    3 deep, xt double-buffered for cross-iteration prefetch.
  - mm1 PSUM drain on DVE (tensor_scalar add-bias + max-0), mm2 drain on
    ScalarE (Identity + bias) -> drains run on different engines.
  - Host: transpose yT back, scatter-add into the [B,S,D] output.
"""

import numpy as np

B, S, D, F, E = 2, 2048, 1024, 4096, 8
T = B * S                      # 4096 tokens
K_TOP = 2
P = 128
NJ = F // P                    # 32 F-tiles
KD = D // P                    # 8  D-tiles
NJ4 = NJ // 4                  # 8  j4 groups (W1 stream granule)
W1COLS = NJ4 * KD * 512        # 16384  (pre-swizzled W1 free size)
W2COLS = NJ * D                # 32768  (pre-swizzled W2 free size)

_CACHE = {}


def _chunks(C, step=512):
    out, c0 = [], 0
    while c0 < C:
        cw = min(step, C - c0)
        out.append((c0, cw))
        c0 += cw
    return out


def _build_program(C, reps=1, loop_n=1, parts="all", ldw="inner"):
    # parts: "all" | "dma" (skip matmuls/acts) | "pe" (hoist input DMAs
    # out of the timing loop)  — probe modes for bottleneck isolation
    import concourse.bass as bass
    import concourse.mybir as mybir
    import concourse.tile as tile
    from concourse import bacc
    from contextlib import ExitStack

    f32 = mybir.dt.float32
    bf16 = mybir.dt.bfloat16

    nc = bacc.Bacc("TRN2", target_bir_lowering=False, debug=False)

    # host-swizzled layouts (see _mk_inmaps):
    #   xt_d[p, k*C + t]          = x[tok_t, k*128 + p]
    #   w1_d[p, (j4*KD + k)*512 + c] = W1[k*128 + p, j4*512 + c]
    #   w2_d[p, k*D + d]          = W2[k*128 + p, d]
    xt_d = nc.dram_tensor("xt", [P, KD * C], bf16, kind="ExternalInput").ap()
    w1_d = nc.dram_tensor("W1r", [P, W1COLS], bf16, kind="ExternalInput").ap()
    w2_d = nc.dram_tensor("W2r", [P, W2COLS], bf16, kind="ExternalInput").ap()
    wcr_d = nc.dram_tensor("wcr", [P, C], bf16, kind="ExternalInput").ap()
    b1_d = nc.dram_tensor("b1t", [P, NJ], f32, kind="ExternalInput").ap()
    b2_d = nc.dram_tensor("b2t", [P, KD], f32, kind="ExternalInput").ap()
    y_d = nc.dram_tensor("yout", [D, C], bf16, kind="ExternalOutput").ap()

    CH = _chunks(C)
    NCH = len(CH)
    GK = KD * 512              # 4096 w1 cols per j4 group

    hoist_dma = (parts == "pe")
    skip_mm = (parts == "dma")

    with tile.TileContext(nc) as tc, ExitStack() as ctx:
        sb = ctx.enter_context(tc.tile_pool(name="sb", bufs=1))
        ps = ctx.enter_context(tc.tile_pool(name="ps", bufs=1, space="PSUM"))

        wcr_t = sb.tile([P, C], bf16, tag="wcr")
        b1_t = sb.tile([P, NJ], f32, tag="b1")
        b2_t = sb.tile([P, KD], f32, tag="b2")
        nc.scalar.dma_start(wcr_t[:], wcr_d[:])
        nc.scalar.dma_start(b1_t[:], b1_d[:])
        nc.scalar.dma_start(b2_t[:], b2_d[:])

        def xt_tile(rep):
            xt = sb.tile([P, KD * C], bf16, tag="xt", bufs=2,
                         name=f"xt_{rep}")
            nc.sync.dma_start(xt[:], xt_d[:])
            return xt

        def w1_tile(rep, j4):
            w1t = sb.tile([P, GK], bf16, tag="w1", bufs=5,
                          name=f"w1_{rep}_{j4}")
            nc.sync.dma_start(w1t[:], w1_d[:, j4 * GK:(j4 + 1) * GK])
            return w1t

        def w2_tile(rep, kg):
            # 4 k-slices per trigger, on the Activation HWDGE queue
            w2t = sb.tile([P, 4 * D], bf16, tag="w2f", bufs=KD,
                          name=f"w2_{rep}_{kg}")
            nc.scalar.dma_start(w2t[:], w2_d[:, kg * 4 * D:(kg + 1) * 4 * D])
            return w2t

        if hoist_dma:
            xt0 = xt_tile(0)
            w2ts0 = [w2_tile(0, kg) for kg in range(KD)]
            w1ts0 = [w1_tile(0, j4) for j4 in range(3)]

        loop_cm = tc.For_i(0, loop_n, 1) if loop_n > 1 else None
        if loop_cm is not None:
            loop_cm.__enter__()

        for rep in range(reps):
            if hoist_dma:
                xt, w2ts = xt0, w2ts0
                w1tiles = [w1ts0[j4 % 3] for j4 in range(NJ4)]
            else:
                xt = xt_tile(rep)
                w2ts = [w2_tile(rep, kg) for kg in range(KD)]
                w1tiles = [w1_tile(rep, j4) for j4 in range(NJ4)]
            h = sb.tile([P, NJ * C], bf16, tag="h", name=f"h_{rep}")

            if skip_mm:
                dummy = sb.tile([P, C], bf16, tag="ot", bufs=2,
                                name=f"dummy_{rep}")
                nc.vector.memset(dummy[:], 0.0)
                for dch in range(KD):
                    nc.scalar.dma_start(y_d[dch * P:(dch + 1) * P, :],
                                        dummy[:])
                continue

            # weight-tile APs in PE execution order; each group's stationary
            # tile is pre-issued (ldweights) during the previous group's
            # matmuls so the PE reorder window hides the load
            def w1ap(j4, jj, k):
                return w1tiles[j4][:, k * 512 + jj * P: k * 512 + (jj + 1) * P]

            def w2ap(dch, k):
                kg, k4 = divmod(k, 4)
                return w2ts[kg][:, k4 * D + dch * P: k4 * D + (dch + 1) * P]

            groups1 = [(j4, jj, k)
                       for j4 in range(NJ4) for jj in range(4)
                       for k in range(KD)]
            groups2 = [(dch, k) for dch in range(KD) for k in range(NJ)]
            nexts = ([w1ap(*g) for g in groups1[1:]] + [w2ap(*groups2[0])]
                     + [w2ap(*g) for g in groups2[1:]] + [None])
            gi = 0

            # --- mm1 + relu:  h[j*C + t] = relu(x @ W1 + b1) ---
            for j4 in range(NJ4):
                for jj in range(4):
                    j = j4 * 4 + jj
                    pcs = [ps.tile([P, 512], f32, tag="mm", bufs=8,
                                   name=f"p1_{rep}_{j}_{ci}")
                           for ci in range(NCH)]
                    for k in range(KD):
                        for ci, (c0, cw) in enumerate(CH):
                            nc.tensor.matmul(
                                pcs[ci][:, :cw],
                                lhsT=w1ap(j4, jj, k),
                                rhs=xt[:, k * C + c0: k * C + c0 + cw],
                                start=(k == 0), stop=(k == KD - 1))
                        if ldw and nexts[gi] is not None and (
                                ldw is True or k < KD - 1):
                            nc.tensor.ldweights(nexts[gi])
                        gi += 1
                    for ci, (c0, cw) in enumerate(CH):
                        # relu(psum + b1) on DVE: add bias then max(,0)
                        nc.vector.tensor_scalar(
                            out=h[:, j * C + c0: j * C + c0 + cw],
                            in0=pcs[ci][:, :cw],
                            scalar1=b1_t[:, j:j + 1], scalar2=0.0,
                            op0=mybir.AluOpType.add,
                            op1=mybir.AluOpType.max)

            # --- mm2:  yT[dch*128+p, t] = sum_j W2[j,d].T @ h[j, t] ---
            # dch outer, chunks innermost: consecutive matmuls share lhsT
            # (walrus dedupes consecutive identical weight loads)
            for dch in range(KD):
                pds = [ps.tile([P, 512], f32, tag="mm", bufs=8,
                               name=f"p2_{rep}_{dch}_{ci}")
                       for ci in range(NCH)]
                for k in range(NJ):
                    for ci, (c0, cw) in enumerate(CH):
                        nc.tensor.matmul(
                            pds[ci][:, :cw],
                            lhsT=w2ap(dch, k),
                            rhs=h[:, k * C + c0: k * C + c0 + cw],
                            start=(k == 0), stop=(k == NJ - 1))
                    if ldw and nexts[gi] is not None and (
                            ldw is True or k < NJ - 1):
                        nc.tensor.ldweights(nexts[gi])
                    gi += 1
                ot = sb.tile([P, C], bf16, tag="ot", bufs=2,
                             name=f"ot_{rep}_{dch}")
                for ci, (c0, cw) in enumerate(CH):
                    nc.scalar.activation(
                        ot[:, c0:c0 + cw], pds[ci][:, :cw],
                        mybir.ActivationFunctionType.Identity,
                        bias=b2_t[:, dch:dch + 1])
                nc.vector.tensor_tensor(
                    out=ot[:], in0=ot[:], in1=wcr_t[:],
                    op=mybir.AluOpType.mult)
                nc.scalar.dma_start(y_d[dch * P:(dch + 1) * P, :], ot[:])

        if loop_cm is not None:
            loop_cm.__exit__(None, None, None)

    nc.compile()
    return nc


def _route(x2, Wg, bg):
    """Host-side top-2 routing in float64 (stable ordering)."""
    gate = x2.astype(np.float64) @ np.asarray(Wg, np.float64) + np.asarray(bg, np.float64)
    part = np.argpartition(-gate, K_TOP - 1, axis=1)[:, :K_TOP]      # [T, 2]
    rows = np.arange(T)[:, None]
    sc = gate[rows, part]                                            # [T, 2]
    sc = sc - sc.max(axis=1, keepdims=True)
    e_sc = np.exp(sc)
    probs = e_sc / e_sc.sum(axis=1, keepdims=True)                   # [T, 2]
    idx_e, w_e, n_e = [], [], []
    for e in range(E):
        mask = part == e                                             # [T, 2]
        tok = np.nonzero(mask.any(axis=1))[0]
        pr = probs[mask]                                             # aligned with tok
        idx_e.append(tok.astype(np.int32))
        w_e.append(pr.astype(np.float32))
        n_e.append(len(tok))
    return idx_e, w_e, n_e


CAP = 1024                     # device per-expert capacity (= T*K_TOP/E);
                               # overflow pairs (~1.5% of FLOPs worst case)
                               # are computed on the host in f32


def _cap_routing(idx_e, w_e, n_e):
    """Split routing into device part (first CAP tokens per expert) and
    host overflow list [(e, toks, wcs)]."""
    idx_c, w_c, n_c, over = [], [], [], []
    for e in range(E):
        n = n_e[e]
        if n > CAP:
            over.append((e, idx_e[e][CAP:], w_e[e][CAP:]))
        idx_c.append(idx_e[e][:CAP])
        w_c.append(w_e[e][:CAP])
        n_c.append(min(n, CAP))
    return idx_c, w_c, n_c, over


def _mk_inmaps(x2, W1, b1, W2, b2, idx_e, w_e, C):
    import ml_dtypes
    bf16 = ml_dtypes.bfloat16
    x2T = np.ascontiguousarray(x2.T)                     # [D, T]
    in_maps = []
    for e in range(E):
        idx = idx_e[e]
        n = len(idx)
        xT = np.zeros((KD, P, C), bf16)                  # [k, p, t]
        xT.reshape(D, C)[:, :n] = x2T[:, idx].astype(bf16)
        wc = np.zeros((C,), np.float32)
        wc[:n] = w_e[e]
        # swizzles: see _build_program layout comments
        w1r = np.ascontiguousarray(
            W1[e].astype(bf16).reshape(KD, P, NJ4, 512)
            .transpose(1, 2, 0, 3)).reshape(P, W1COLS)
        w2r = np.ascontiguousarray(
            W2[e].astype(bf16).reshape(NJ, P, D)
            .transpose(1, 0, 2)).reshape(P, W2COLS)
        in_maps.append({
            "xt": np.ascontiguousarray(xT.transpose(1, 0, 2)).reshape(P, KD * C),
            "W1r": w1r,
            "W2r": w2r,
            "wcr": np.ascontiguousarray(np.broadcast_to(wc, (P, C))).astype(bf16),
            "b1t": np.ascontiguousarray(b1[e].reshape(NJ, P).T),
            "b2t": np.ascontiguousarray(b2[e].reshape(KD, P).T),
        })
    return in_maps


def kernel(x, W1, b1, W2, b2, Wg, bg, num_experts_per_token):
    from concourse.bass_utils import run_bass_kernel_spmd

    x2 = np.ascontiguousarray(np.asarray(x, np.float32).reshape(T, D))
    W1 = np.asarray(W1, np.float32)
    b1 = np.asarray(b1, np.float32)
    W2 = np.asarray(W2, np.float32)
    b2 = np.asarray(b2, np.float32)

    if int(num_experts_per_token) != K_TOP:
        # unexpected top-k: correct numpy slow path
        gate = x2.astype(np.float64) @ np.asarray(Wg, np.float64) + np.asarray(bg, np.float64)
        k = int(num_experts_per_token)
        part = np.argsort(-gate, axis=1)[:, :k]
        sc = gate[np.arange(T)[:, None], part]
        sc = sc - sc.max(axis=1, keepdims=True)
        pr = np.exp(sc); pr /= pr.sum(axis=1, keepdims=True)
        out = np.zeros((T, D), np.float32)
        for e in range(E):
            mask = part == e
            tok = np.nonzero(mask.any(axis=1))[0]
            w = pr[mask].astype(np.float32)
            hcur = np.maximum(x2[tok] @ W1[e] + b1[e], 0.0)
            out[tok] += w[:, None] * (hcur @ W2[e] + b2[e])
        return out.reshape(B, S, D)

    idx_e, w_e, n_e, over = _cap_routing(*_route(x2, Wg, bg))
    C = max((max(n_e) + 7) // 8 * 8, 64)

    if C not in _CACHE:
        _CACHE[C] = _build_program(C)
    nc = _CACHE[C]

    in_maps = _mk_inmaps(x2, W1, b1, W2, b2, idx_e, w_e, C)
    res = run_bass_kernel_spmd(nc, in_maps, list(range(E)))

    out = np.zeros((T, D), np.float32)
    for e in range(E):
        n = n_e[e]
        yT = np.asarray(res.results[e]["yout"], np.float32)  # [D, C] (bf16 on device)
        out[idx_e[e]] += yT[:, :n].T
    # host-side overflow pass (tokens beyond per-expert capacity)
    for e, toks, wcs in over:
        ho = np.maximum(x2[toks] @ W1[e] + b1[e], 0.0)
        out[toks] += wcs[:, None] * (ho @ W2[e] + b2[e])
    return out.reshape(B, S, D)


# revision 34
# speedup vs baseline: 1.4369x; 1.2040x over previous
"""Expert-parallel MoE feed-forward for Trainium2 (8 NeuronCores).

Strategy:
  - Host: gate + top-2 routing (0.02% of FLOPs), per-expert token lists,
    gather + transpose + bf16-cast of each expert's tokens -> xT [D, C].
    Expert e is owned by core e.  C = max expert load, rounded up to 8
    (1080 for the reference input); the program is compiled per C and
    cached.  All device inputs are pre-swizzled partition-major on the
    host so every DMA is one large 2D transfer with multi-KB lines.
  - Device (same SPMD program on all 8 cores), all matmuls bf16 (fp32
    PSUM accumulate; rel err ~7e-3 << 2e-2 tolerance):
      mm1: h[f, t]  = relu(W1[k,f-slice].T @ xT[k, t] + b1)   (acc over k)
      mm2: yT[d, t] = W2[j,d-slice].T @ h[j, t] + b2          (acc over j)
      yT *= wc[t];  store yT [D, C] bf16 (host upcasts).
    No device gather / transposes: PE runs a pure matmul stream
    (~2*256*C rows; bf16 moving streams ~2 elem/cycle).
  - Weight-load hiding: matmuls sharing a stationary tile are emitted
    consecutively (walrus dedupes consecutive identical weight loads),
    and the next tile is pre-issued via nc.tensor.ldweights between
    accumulation steps (ldw="inner") so the PE reorder window loads it
    during the current matmuls: measured 267us -> 194us (uncapped C).
    Early wedge suspicion about this mode was retired by a dedicated
    stress test: 40 single execs + 676 loop iterations, 0 hangs,
    correct output throughout (stress_ldw.py).
  - DMA triggers split across the two HWDGE issuing engines: SP carries
    the mm1-critical xt + W1 stream in consumption order, Activation
    carries W2 + y-out.  17 input triggers per iteration total; w1 ring
    5 deep, xt double-buffered for cross-iteration prefetch.
  - mm1 PSUM drain on DVE (tensor_scalar add-bias + max-0), mm2 drain on
    ScalarE (Identity + bias) -> drains run on different engines.
  - Host: transpose yT back, scatter-add into the [B,S,D] output.
"""

import numpy as np

B, S, D, F, E = 2, 2048, 1024, 4096, 8
T = B * S                      # 4096 tokens
K_TOP = 2
P = 128
NJ = F // P                    # 32 F-tiles
KD = D // P                    # 8  D-tiles
NJ4 = NJ // 4                  # 8  j4 groups (W1 stream granule)
W1COLS = NJ4 * KD * 512        # 16384  (pre-swizzled W1 free size)
W2COLS = NJ * D                # 32768  (pre-swizzled W2 free size)

_CACHE = {}


def _chunks(C, step=512):
    out, c0 = [], 0
    while c0 < C:
        cw = min(step, C - c0)
        out.append((c0, cw))
        c0 += cw
    return out


def _build_program(C, reps=1, loop_n=1, parts="all", ldw="inner"):
    # parts: "all" | "dma" (skip matmuls/acts) | "pe" (hoist input DMAs
    # out of the timing loop)  — probe modes for bottleneck isolation
    import concourse.bass as bass
    import concourse.mybir as mybir
    import concourse.tile as tile
    from concourse import bacc
    from contextlib import ExitStack

    f32 = mybir.dt.float32
    bf16 = mybir.dt.bfloat16

    nc = bacc.Bacc("TRN2", target_bir_lowering=False, debug=False)

    # host-swizzled layouts (see _mk_inmaps):
    #   xt_d[p, k*C + t]          = x[tok_t, k*128 + p]
    #   w1_d[p, (j4*KD + k)*512 + c] = W1[k*128 + p, j4*512 + c]
    #   w2_d[p, k*D + d]          = W2[k*128 + p, d]
    xt_d = nc.dram_tensor("xt", [P, KD * C], bf16, kind="ExternalInput").ap()
    w1_d = nc.dram_tensor("W1r", [P, W1COLS], bf16, kind="ExternalInput").ap()
    w2_d = nc.dram_tensor("W2r", [P, W2COLS], bf16, kind="ExternalInput").ap()
    wcr_d = nc.dram_tensor("wcr", [P, C], bf16, kind="ExternalInput").ap()
    b1_d = nc.dram_tensor("b1t", [P, NJ], f32, kind="ExternalInput").ap()
    b2_d = nc.dram_tensor("b2t", [P, KD], f32, kind="ExternalInput").ap()
    y_d = nc.dram_tensor("yout", [D, C], bf16, kind="ExternalOutput").ap()

    CH = _chunks(C)
    NCH = len(CH)
    GK = KD * 512              # 4096 w1 cols per j4 group

    hoist_dma = (parts == "pe")
    skip_mm = (parts == "dma")

    with tile.TileContext(nc) as tc, ExitStack() as ctx:
        sb = ctx.enter_context(tc.tile_pool(name="sb", bufs=1))
        ps = ctx.enter_context(tc.tile_pool(name="ps", bufs=1, space="PSUM"))

        wcr_t = sb.tile([P, C], bf16, tag="wcr")
        b1_t = sb.tile([P, NJ], f32, tag="b1")
        b2_t = sb.tile([P, KD], f32, tag="b2")
        nc.scalar.dma_start(wcr_t[:], wcr_d[:])
        nc.scalar.dma_start(b1_t[:], b1_d[:])
        nc.scalar.dma_start(b2_t[:], b2_d[:])

        def xt_tile(rep):
            xt = sb.tile([P, KD * C], bf16, tag="xt", bufs=2,
                         name=f"xt_{rep}")
            nc.sync.dma_start(xt[:], xt_d[:])
            return xt

        def w1_tile(rep, j4):
            # alternate j4 groups across the two HWDGE queues: each queue
            # gets 2x the delivery window for its just-in-time 1MB bursts
            w1t = sb.tile([P, GK], bf16, tag="w1", bufs=5,
                          name=f"w1_{rep}_{j4}")
            eng = nc.sync if j4 % 2 == 0 else nc.scalar
            eng.dma_start(w1t[:], w1_d[:, j4 * GK:(j4 + 1) * GK])
            return w1t

        def w2_tile(rep, kg):
            # 4 k-slices per trigger, on the Activation HWDGE queue
            w2t = sb.tile([P, 4 * D], bf16, tag="w2f", bufs=KD,
                          name=f"w2_{rep}_{kg}")
            nc.scalar.dma_start(w2t[:], w2_d[:, kg * 4 * D:(kg + 1) * 4 * D])
            return w2t

        # HAM warm-up: ~3.4us of dummy matmuls at program start, overlapping
        # the initial xt/w1 DMA window, so the PE reaches K=8/8 (2.4 GHz)
        # before the real matmul stream begins (single-execution win only)
        warm = sb.tile([P, 512], bf16, tag="warm")
        nc.vector.memset(warm[:], 0.0)
        wp = ps.tile([P, 512], f32, tag="mm", bufs=8, name="warm_ps")
        for i in range(32):
            nc.tensor.matmul(wp[:], lhsT=warm[:, :P], rhs=warm[:],
                             start=(i == 0), stop=(i == 31))
        nc.vector.tensor_copy(warm[:], wp[:])

        if hoist_dma:
            xt0 = xt_tile(0)
            w2ts0 = [w2_tile(0, kg) for kg in range(KD)]
            w1ts0 = [w1_tile(0, j4) for j4 in range(3)]

        loop_cm = tc.For_i(0, loop_n, 1) if loop_n > 1 else None
        if loop_cm is not None:
            loop_cm.__enter__()

        for rep in range(reps):
            if hoist_dma:
                xt, w2ts = xt0, w2ts0
                w1tiles = [w1ts0[j4 % 3] for j4 in range(NJ4)]
            else:
                xt = xt_tile(rep)
                w1tiles = [w1_tile(rep, j4) for j4 in range(NJ4)]
                w2ts = [w2_tile(rep, kg) for kg in range(KD)]
            h = sb.tile([P, NJ * C], bf16, tag="h", name=f"h_{rep}")

            if skip_mm:
                dummy = sb.tile([P, C], bf16, tag="ot", bufs=2,
                                name=f"dummy_{rep}")
                nc.vector.memset(dummy[:], 0.0)
                for dch in range(KD):
                    nc.scalar.dma_start(y_d[dch * P:(dch + 1) * P, :],
                                        dummy[:])
                continue

            # weight-tile APs in PE execution order; each group's stationary
            # tile is pre-issued (ldweights) during the previous group's
            # matmuls so the PE reorder window hides the load
            def w1ap(j4, jj, k):
                return w1tiles[j4][:, k * 512 + jj * P: k * 512 + (jj + 1) * P]

            def w2ap(dch, k):
                kg, k4 = divmod(k, 4)
                return w2ts[kg][:, k4 * D + dch * P: k4 * D + (dch + 1) * P]

            groups1 = [(j4, jj, k)
                       for j4 in range(NJ4) for jj in range(4)
                       for k in range(KD)]
            groups2 = [(dch, k) for dch in range(KD) for k in range(NJ)]
            nexts = ([w1ap(*g) for g in groups1[1:]] + [w2ap(*groups2[0])]
                     + [w2ap(*g) for g in groups2[1:]] + [None])
            gi = 0

            # --- mm1 + relu:  h[j*C + t] = relu(x @ W1 + b1) ---
            for j4 in range(NJ4):
                for jj in range(4):
                    j = j4 * 4 + jj
                    pcs = [ps.tile([P, 512], f32, tag="mm", bufs=8,
                                   name=f"p1_{rep}_{j}_{ci}")
                           for ci in range(NCH)]
                    for k in range(KD):
                        for ci, (c0, cw) in enumerate(CH):
                            nc.tensor.matmul(
                                pcs[ci][:, :cw],
                                lhsT=w1ap(j4, jj, k),
                                rhs=xt[:, k * C + c0: k * C + c0 + cw],
                                start=(k == 0), stop=(k == KD - 1))
                        if ldw and nexts[gi] is not None and (
                                ldw is True or k < KD - 1):
                            nc.tensor.ldweights(nexts[gi])
                        gi += 1
                    for ci, (c0, cw) in enumerate(CH):
                        # relu(psum + b1) on DVE: add bias then max(,0)
                        nc.vector.tensor_scalar(
                            out=h[:, j * C + c0: j * C + c0 + cw],
                            in0=pcs[ci][:, :cw],
                            scalar1=b1_t[:, j:j + 1], scalar2=0.0,
                            op0=mybir.AluOpType.add,
                            op1=mybir.AluOpType.max)

            # --- mm2:  yT[dch*128+p, t] = sum_j W2[j,d].T @ h[j, t] ---
            # dch outer, chunks innermost: consecutive matmuls share lhsT
            # (walrus dedupes consecutive identical weight loads)
            for dch in range(KD):
                pds = [ps.tile([P, 512], f32, tag="mm", bufs=8,
                               name=f"p2_{rep}_{dch}_{ci}")
                       for ci in range(NCH)]
                for k in range(NJ):
                    for ci, (c0, cw) in enumerate(CH):
                        nc.tensor.matmul(
                            pds[ci][:, :cw],
                            lhsT=w2ap(dch, k),
                            rhs=h[:, k * C + c0: k * C + c0 + cw],
                            start=(k == 0), stop=(k == NJ - 1))
                    if ldw and nexts[gi] is not None and (
                            ldw is True or k < NJ - 1):
                        nc.tensor.ldweights(nexts[gi])
                    gi += 1
                ot = sb.tile([P, C], bf16, tag="ot", bufs=2,
                             name=f"ot_{rep}_{dch}")
                for ci, (c0, cw) in enumerate(CH):
                    nc.scalar.activation(
                        ot[:, c0:c0 + cw], pds[ci][:, :cw],
                        mybir.ActivationFunctionType.Identity,
                        bias=b2_t[:, dch:dch + 1])
                nc.vector.tensor_tensor(
                    out=ot[:], in0=ot[:], in1=wcr_t[:],
                    op=mybir.AluOpType.mult)
                nc.scalar.dma_start(y_d[dch * P:(dch + 1) * P, :], ot[:])

        if loop_cm is not None:
            loop_cm.__exit__(None, None, None)

    nc.compile()
    return nc


def _route(x2, Wg, bg):
    """Host-side top-2 routing in float64 (stable ordering)."""
    gate = x2.astype(np.float64) @ np.asarray(Wg, np.float64) + np.asarray(bg, np.float64)
    part = np.argpartition(-gate, K_TOP - 1, axis=1)[:, :K_TOP]      # [T, 2]
    rows = np.arange(T)[:, None]
    sc = gate[rows, part]                                            # [T, 2]
    sc = sc - sc.max(axis=1, keepdims=True)
    e_sc = np.exp(sc)
    probs = e_sc / e_sc.sum(axis=1, keepdims=True)                   # [T, 2]
    idx_e, w_e, n_e = [], [], []
    for e in range(E):
        mask = part == e                                             # [T, 2]
        tok = np.nonzero(mask.any(axis=1))[0]
        pr = probs[mask]                                             # aligned with tok
        idx_e.append(tok.astype(np.int32))
        w_e.append(pr.astype(np.float32))
        n_e.append(len(tok))
    return idx_e, w_e, n_e


CAP = 1024                     # device per-expert capacity (= T*K_TOP/E);
                               # overflow pairs (~1.5% of FLOPs worst case)
                               # are computed on the host in f32


def _cap_routing(idx_e, w_e, n_e):
    """Split routing into device part (first CAP tokens per expert) and
    host overflow list [(e, toks, wcs)]."""
    idx_c, w_c, n_c, over = [], [], [], []
    for e in range(E):
        n = n_e[e]
        if n > CAP:
            over.append((e, idx_e[e][CAP:], w_e[e][CAP:]))
        idx_c.append(idx_e[e][:CAP])
        w_c.append(w_e[e][:CAP])
        n_c.append(min(n, CAP))
    return idx_c, w_c, n_c, over


def _mk_inmaps(x2, W1, b1, W2, b2, idx_e, w_e, C):
    import ml_dtypes
    bf16 = ml_dtypes.bfloat16
    x2T = np.ascontiguousarray(x2.T)                     # [D, T]
    in_maps = []
    for e in range(E):
        idx = idx_e[e]
        n = len(idx)
        xT = np.zeros((KD, P, C), bf16)                  # [k, p, t]
        xT.reshape(D, C)[:, :n] = x2T[:, idx].astype(bf16)
        wc = np.zeros((C,), np.float32)
        wc[:n] = w_e[e]
        # swizzles: see _build_program layout comments
        w1r = np.ascontiguousarray(
            W1[e].astype(bf16).reshape(KD, P, NJ4, 512)
            .transpose(1, 2, 0, 3)).reshape(P, W1COLS)
        w2r = np.ascontiguousarray(
            W2[e].astype(bf16).reshape(NJ, P, D)
            .transpose(1, 0, 2)).reshape(P, W2COLS)
        in_maps.append({
            "xt": np.ascontiguousarray(xT.transpose(1, 0, 2)).reshape(P, KD * C),
            "W1r": w1r,
            "W2r": w2r,
            "wcr": np.ascontiguousarray(np.broadcast_to(wc, (P, C))).astype(bf16),
            "b1t": np.ascontiguousarray(b1[e].reshape(NJ, P).T),
            "b2t": np.ascontiguousarray(b2[e].reshape(KD, P).T),
        })
    return in_maps


def kernel(x, W1, b1, W2, b2, Wg, bg, num_experts_per_token):
    from concourse.bass_utils import run_bass_kernel_spmd

    x2 = np.ascontiguousarray(np.asarray(x, np.float32).reshape(T, D))
    W1 = np.asarray(W1, np.float32)
    b1 = np.asarray(b1, np.float32)
    W2 = np.asarray(W2, np.float32)
    b2 = np.asarray(b2, np.float32)

    if int(num_experts_per_token) != K_TOP:
        # unexpected top-k: correct numpy slow path
        gate = x2.astype(np.float64) @ np.asarray(Wg, np.float64) + np.asarray(bg, np.float64)
        k = int(num_experts_per_token)
        part = np.argsort(-gate, axis=1)[:, :k]
        sc = gate[np.arange(T)[:, None], part]
        sc = sc - sc.max(axis=1, keepdims=True)
        pr = np.exp(sc); pr /= pr.sum(axis=1, keepdims=True)
        out = np.zeros((T, D), np.float32)
        for e in range(E):
            mask = part == e
            tok = np.nonzero(mask.any(axis=1))[0]
            w = pr[mask].astype(np.float32)
            hcur = np.maximum(x2[tok] @ W1[e] + b1[e], 0.0)
            out[tok] += w[:, None] * (hcur @ W2[e] + b2[e])
        return out.reshape(B, S, D)

    idx_e, w_e, n_e, over = _cap_routing(*_route(x2, Wg, bg))
    C = max((max(n_e) + 7) // 8 * 8, 64)

    if C not in _CACHE:
        _CACHE[C] = _build_program(C)
    nc = _CACHE[C]

    in_maps = _mk_inmaps(x2, W1, b1, W2, b2, idx_e, w_e, C)
    res = run_bass_kernel_spmd(nc, in_maps, list(range(E)))

    out = np.zeros((T, D), np.float32)
    for e in range(E):
        n = n_e[e]
        yT = np.asarray(res.results[e]["yout"], np.float32)  # [D, C] (bf16 on device)
        out[idx_e[e]] += yT[:, :n].T
    # host-side overflow pass (tokens beyond per-expert capacity)
    for e, toks, wcs in over:
        ho = np.maximum(x2[toks] @ W1[e] + b1[e], 0.0)
        out[toks] += wcs[:, None] * (ho @ W2[e] + b2[e])
    return out.reshape(B, S, D)
